# revision 11
# baseline (speedup 1.0000x reference)
"""MultiHeadGraphAttention Trainium2 kernel.

Data-parallel over batch: core b computes batch element b (B=8, 8 cores).

Per-core math (one batch element, N=2048 nodes, U=256 units, H=8 heads, d=32):
  Q = x Wq, K = x Wk, V = x Wv                      (projections)
  sT[k,q]  = sum_d KT[d,k] QT[d,q]                  (scores, transposed layout)
  e        = exp(sT/sqrt(d)) * adjT                 (masked exp)
  ctxT[d,q] = sum_k V[k,d] e[k,q]  ;  Z[q] = sum_k e[k,q]
  out      = (ctxT/Z).T @ Wo + bo

v5 design notes:
  - scores kept transposed [k(part), q(free)]; per (kb, head-group) one
    [128, 4*512] f32 score tile (4 banks, one full bank per head: concurrent
    row-banded matmuls must not share a PSUM bank), one FD=2048 ACTIVATE.
    spool bufs=1: the 4-wide score wave serializes with ACT on those banks,
    but keeps all PE waves 4-wide (the cold-clock PE needs the concurrency).
  - adjacency mask arrives from host as bf16 (values 0/1 exact): mask DMA is
    8MB not 16MB and there is no on-device int32->bf16 conversion pass.
  - Z rows are produced PRE-BROADCAST (ones stationary [128,32] -> Z lands
    replicated across each head's 32 partitions; same matmul cost). The
    per-qc normalization is then just reciprocal+multiply, no DMA broadcast.
  - out projection in bf16 per 128-row block with immediate DMA (no tail).
PSUM: spool 4 banks + cps 2 + zps 2 = 8.
"""

import sys

for p in ("/opt/trn_rl_repo",):
    if p not in sys.path:
        sys.path.insert(0, p)

from contextlib import ExitStack

import numpy as np
import ml_dtypes

import concourse.bass as bass
import concourse.mybir as mybir
import concourse.tile as tile
from concourse import bacc
from concourse.bass_utils import run_bass_kernel_spmd

B, N, U, H, D = 8, 2048, 256, 8, 32
NB = N // 128          # 16 key blocks of 128
QC = 4                 # q chunks
QW = N // QC           # 512 q per chunk
SCALE = 1.0 / np.sqrt(np.float32(D))

f32 = mybir.dt.float32
bf16 = mybir.dt.bfloat16
EXP = mybir.ActivationFunctionType.Exp
MULT = mybir.AluOpType.mult


def build_program():
    nc = bacc.Bacc("TRN2", target_bir_lowering=False, debug=False,
                   enable_asserts=False, num_devices=B)

    xT_d = nc.dram_tensor("xT", [U, N], bf16, kind="ExternalInput").ap()
    mT_d = nc.dram_tensor("mT", [N, N], bf16, kind="ExternalInput").ap()
    wq_d = nc.dram_tensor("Wq", [U, U], bf16, kind="ExternalInput").ap()
    wk_d = nc.dram_tensor("Wk", [U, U], bf16, kind="ExternalInput").ap()
    wv_d = nc.dram_tensor("Wv", [U, U], bf16, kind="ExternalInput").ap()
    wo_d = nc.dram_tensor("Wo", [U, U], bf16, kind="ExternalInput").ap()
    bo_d = nc.dram_tensor("bo", [U], f32, kind="ExternalInput").ap()
    out_d = nc.dram_tensor("out", [N, U], f32, kind="ExternalOutput").ap()

    with tile.TileContext(nc) as tc:
        with ExitStack() as ctx:
            kernel_body(ctx, tc, xT_d, mT_d, wq_d, wk_d, wv_d, wo_d,
                        bo_d, out_d)
    nc.compile()
    return nc


def kernel_body(ctx, tc, xT_d, mT_d, wq_d, wk_d, wv_d, wo_d, bo_d, out_d):
    nc = tc.nc
    persist = ctx.enter_context(tc.tile_pool(name="persist", bufs=1))
    stage = ctx.enter_context(tc.tile_pool(name="stage", bufs=4))
    epool = ctx.enter_context(tc.tile_pool(name="epool", bufs=3))
    npool = ctx.enter_context(tc.tile_pool(name="npool", bufs=4))
    spool = ctx.enter_context(tc.tile_pool(name="spool", bufs=1, space="PSUM"))
    cpool = ctx.enter_context(tc.tile_pool(name="cpool", bufs=2, space="PSUM"))
    zpool = ctx.enter_context(tc.tile_pool(name="zpool", bufs=2, space="PSUM"))

    # ---- persistent SBUF tensors -------------------------------------------
    qT = [persist.tile([128, N], bf16, tag=f"qT{c}", name=f"qT{c}") for c in range(2)]
    kT = [persist.tile([128, N], bf16, tag=f"kT{c}", name=f"kT{c}") for c in range(2)]
    v_sb = persist.tile([128, NB * U], bf16, tag="v")
    m_sb = persist.tile([128, NB * N], bf16, tag="m")
    w_sb = {}
    for nm, dram in (("wq", wq_d), ("wk", wk_d), ("wv", wv_d), ("wo", wo_d)):
        w_sb[nm] = persist.tile([128, 2 * U], bf16, tag=nm, name=nm)
        for c in range(2):
            nc.scalar.dma_start(w_sb[nm][:, c * U:(c + 1) * U],
                                dram[c * 128:(c + 1) * 128, :])
    bo_sb = persist.tile([1, U], f32, tag="bo")
    nc.scalar.dma_start(bo_sb[:], bo_d.rearrange("(o n) -> o n", o=1))
    ones_bf = persist.tile([128, 32], bf16, tag="ones_bf")
    nc.vector.memset(ones_bf[:], 1.0)
    ones_f = persist.tile([1, 128], f32, tag="ones_f")
    nc.vector.memset(ones_f[:], 1.0)
    ctxn = [persist.tile([128, N], bf16, tag=f"ctxn{c}", name=f"ctxn{c}")
            for c in range(2)]

    # ---- input staging ------------------------------------------------------
    xT = [stage.tile([128, N], bf16, tag="stage", name=f"xT{c}") for c in range(2)]
    for c in range(2):
        nc.scalar.dma_start(xT[c][:], xT_d[c * 128:(c + 1) * 128, :])
    for kb in range(NB):
        eng = nc.sync if kb % 2 == 0 else nc.gpsimd
        eng.dma_start(m_sb[:, kb * N:(kb + 1) * N],
                      mT_d[kb * 128:(kb + 1) * 128, :])

    # ---- projections --------------------------------------------------------
    for w, dst in (("wq", qT), ("wk", kT)):
        for mo in range(2):           # output chunk (128 rows of QT/KT)
            ps = spool.tile([128, 4 * QW], f32, tag="s")
            for nn in range(QC):      # 512-wide slices (full bank each)
                for kc in range(2):
                    nc.tensor.matmul(
                        ps[:, nn * QW:(nn + 1) * QW],
                        w_sb[w][:, (kc * 2 + mo) * 128:(kc * 2 + mo + 1) * 128],
                        xT[kc][:, nn * QW:(nn + 1) * QW],
                        start=(kc == 0), stop=(kc == 1))
            nc.scalar.copy(dst[mo][:], ps[:])
    for kb in range(NB):              # V = x @ Wv, natural layout, bf16
        ps = cpool.tile([128, QW], f32, tag="c")
        for kc in range(2):
            nc.tensor.matmul(
                ps[:, :U],
                xT[kc][:, kb * 128:(kb + 1) * 128],
                w_sb["wv"][:, kc * U:(kc + 1) * U],
                start=(kc == 0), stop=(kc == 1))
        nc.vector.tensor_copy(v_sb[:, kb * U:(kb + 1) * U], ps[:, :U])

    # ---- main attention loop ------------------------------------------------
    for qc in range(QC):
        qs = qc * QW
        cps = [cpool.tile([128, QW], f32, tag="c", name=f"cps{g}_{qc}")
               for g in range(2)]
        zps = [zpool.tile([128, QW], f32, tag="z", name=f"zps{g}_{qc}")
               for g in range(2)]
        for kb in range(NB):
            for g in range(2):        # head groups: g=0 -> h0-3, g=1 -> h4-7
                sps = spool.tile([128, 4 * QW], f32, tag="s")
                for j in range(4):
                    nc.tensor.matmul(
                        sps[:, j * QW:(j + 1) * QW],
                        kT[g][32 * j:32 * (j + 1), kb * 128:(kb + 1) * 128],
                        qT[g][32 * j:32 * (j + 1), qs:qs + QW],
                        start=True, stop=True,
                        tile_position=(32 * j, 0))
                e = epool.tile([128, 4 * QW], bf16, tag="e")
                nc.scalar.activation(e[:], sps[:], EXP, scale=float(SCALE))
                me = m_sb[:, kb * N + qs:kb * N + qs + QW]
                nc.vector.tensor_tensor(
                    e.rearrange("p (j q) -> p j q", j=4),
                    e.rearrange("p (j q) -> p j q", j=4),
                    me.unsqueeze(1).broadcast_to([128, 4, QW]), MULT)
                for j in range(4):
                    ej = e[:, j * QW:(j + 1) * QW]
                    nc.tensor.matmul(
                        cps[g][32 * j:32 * (j + 1), :],
                        v_sb[:, kb * U + (4 * g + j) * D:
                             kb * U + (4 * g + j + 1) * D],
                        ej, start=(kb == 0), stop=(kb == NB - 1),
                        tile_position=(0, 32 * j))
                for j in range(4):
                    ej = e[:, j * QW:(j + 1) * QW]
                    nc.tensor.matmul(
                        zps[g][32 * j:32 * (j + 1), :],
                        ones_bf[:], ej,
                        start=(kb == 0), stop=(kb == NB - 1),
                        tile_position=(0, 32 * j))
        # normalize: Z is pre-broadcast across each head's 32 partitions
        for g in range(2):
            zrec = npool.tile([128, QW], f32, tag="n", name=f"zrec{g}_{qc}")
            nc.vector.reciprocal_approx_fast(zrec[:], zps[g][:])
            nc.vector.tensor_tensor(ctxn[g][:, qs:qs + QW], cps[g][:],
                                    zrec[:], MULT)

    # ---- out projection + store, all 128-row blocks at the end -------------
    for qb in range(N // 128):
        ops = zpool.tile([128, QW], f32, tag="z", name=f"ops{qb}")
        for c in range(2):
            nc.tensor.matmul(
                ops[:, :U],
                ctxn[c][:, qb * 128:(qb + 1) * 128],
                w_sb["wo"][:, c * U:(c + 1) * U],
                start=(c == 0), stop=False)
        nc.tensor.matmul(ops[:, :U], ones_f[:], bo_sb[:],
                         start=False, stop=True, skip_group_check=True)
        ob = stage.tile([128, U], f32, tag="ostage", name=f"ob{qb}")
        nc.vector.tensor_copy(ob[:], ops[:, :U])
        nc.sync.dma_start(out_d[qb * 128:(qb + 1) * 128, :], ob[:])


_CACHED = None


def _get_program():
    global _CACHED
    if _CACHED is None:
        _CACHED = build_program()
    return _CACHED


def kernel(node_features, adjacency_matrix, Wq, Wk, Wv, Wo, bo, **run_kwargs):
    nc = _get_program()
    bf = ml_dtypes.bfloat16
    xT = np.ascontiguousarray(np.transpose(node_features, (0, 2, 1))).astype(bf)
    adjT = np.transpose(adjacency_matrix, (0, 2, 1))
    mT = np.ascontiguousarray(adjT).astype(bf)
    in_maps = []
    for b in range(B):
        in_maps.append({
            "xT": xT[b], "mT": mT[b],
            "Wq": np.asarray(Wq, bf), "Wk": np.asarray(Wk, bf),
            "Wv": np.asarray(Wv, bf), "Wo": np.asarray(Wo, bf),
            "bo": np.asarray(bo, np.float32),
        })
    res = run_bass_kernel_spmd(nc, in_maps, core_ids=list(range(B)), **run_kwargs)
    out = np.stack([res.results[b]["out"] for b in range(B)], axis=0)
    kernel.last_results = res
    return out
